# revision 45
# baseline (speedup 1.0000x reference)
"""Trainium2 Bass kernel for nn_CrossAttFA (retrieval_knn).

Math (reference):
  q = W @ x1 (1x1 conv, per-view), k = W @ x2, v = x3
  Q = l2norm(unfold3x3(q) regrouped to [b, L, 1800]), K likewise
  attn = Q @ K^T  [b, L, L];  idx = argmax(attn, -1)
  out = fold3x3(gather rows of unfold(v) by idx)

Device formulation (per batch b): fold the horizontal patch shift dx into
channels: qp[(a,c,dx), u] = q[a,c, uy-1, x+dx-1] over a vertically padded
50x48 pixel grid (u = uy*48+x, uy in [0,50)).  Then with
  S[u, v] = sum_ch qp[ch, u] * kp[ch, v]           (600-dim contraction)
  attn[n, m] = sum_{dy in 0..2} S[n + 48*dy, m + 48*dy]
and the column scale rk[m] = 1/||K_m||, argmax_m attn[n,m]*rk[m] equals
the reference argmax (row scale does not affect argmax).

Precision scheme: all device arithmetic is bf16 (4x faster matmuls, 2x
faster DVE adds).  bf16 ranking is approximate, so the device returns
top-8 *candidate windows* (win=8 cols) per attn row and the host
rescores those <=64 candidate columns exactly in fp64.  CPU analysis of
the fixed input distribution shows the true argmax is always within the
top-3 windows, so top-8 has a wide safety margin.  To make the top-8
window extraction tie-proof, window maxes are upcast to fp32 and the
window index is OR-ed into the low mantissa bits (bf16 upcast leaves the
low 16 bits zero), making all values distinct; max8 alone then returns
value+index in one payload and the host decodes the index bits.

Sharding: 8 cores = 2 batches x 4 row-slabs of 576 attention rows each.
Host does the 1x1 conv + layout prep and the rescore/gather/fold
epilogue.
"""
import sys

sys.path.insert(0, '/opt/trn_rl_repo')
import numpy as np

B, C, AH, AW, H, W_ = 2, 64, 5, 5, 48, 48
A = AH * AW                  # 25 views
L = H * W_                   # 2304 pixels
CH = A * C // 8 * 3          # 600 channels (a, c_out=8, dx=3)
CO = 8                       # conv output channels
CHP = 640                    # padded to 5 K-chunks of 128
UR = 2400                    # padded u-grid rows (50 x 48)
NCORES = 8
SLAB = L // 4                # 576 attn rows per core
USLAB = SLAB + 96            # S rows needed per core (incl. +48,+96 halo)
NT = 480                     # matmul moving free dim (psum bank = 512 fp32)
WIN = 8                      # candidate window width
NW = L // WIN                # 288 windows per attn row

_PROG = None


def _build_program():
    import concourse.bass as bass
    import concourse.bacc as bacc
    import concourse.mybir as mybir
    from concourse.tile import TileContext

    nc = bacc.Bacc('TRN2', target_bir_lowering=False, debug=False,
                   num_devices=NCORES)
    qpT_in = nc.declare_dram_parameter("qpT", [128, 5 * USLAB],
                                       mybir.dt.bfloat16, isOutput=False)
    kpT_in = nc.declare_dram_parameter("kpT", [128, 5 * UR],
                                       mybir.dt.bfloat16, isOutput=False)
    rk_in = nc.declare_dram_parameter("rk", [1, L],
                                      mybir.dt.bfloat16, isOutput=False)
    iota_in = nc.declare_dram_parameter("iota", [1, NW],
                                        mybir.dt.uint32, isOutput=False)

    mx_out = nc.declare_dram_parameter("mx", [5, 128, 8],
                                       mybir.dt.float32, isOutput=True)

    n_sp = (USLAB + 127) // 128          # 6 S-row tiles (last is 32 rows)
    sp_rows = [min(128, USLAB - 128 * t) for t in range(n_sp)]
    n_at = (SLAB + 127) // 128           # 5 attn tiles (last is 64 rows)
    at_rows = [min(128, SLAB - 128 * t) for t in range(n_at)]

    with TileContext(nc) as tc, nc.allow_low_precision(
            reason="bf16 candidate ranking; host rescores exactly"):
        with tc.tile_pool(name="inp", bufs=1) as inp, \
             tc.tile_pool(name="sp", bufs=1) as spp, \
             tc.tile_pool(name="stg", bufs=3) as stg, \
             tc.tile_pool(name="acc", bufs=3) as accp, \
             tc.tile_pool(name="res", bufs=2) as resp, \
             tc.tile_pool(name="ps", bufs=1, space="PSUM") as psp:

            kp_t = inp.tile([128, 5 * UR], mybir.dt.bfloat16, tag="kp")
            qp_t = inp.tile([128, 5 * USLAB], mybir.dt.bfloat16, tag="qp")
            rk1_t = inp.tile([1, L], mybir.dt.bfloat16, tag="rk1")
            iota1_t = inp.tile([1, NW], mybir.dt.uint32, tag="iota1")
            rk_t = inp.tile([128, L], mybir.dt.bfloat16, tag="rk")
            iota_t = inp.tile([128, NW], mybir.dt.uint32, tag="iota")
            # progressive input transfers: small early pieces so the PE
            # starts immediately, kc-granular later ones so no matmul waits
            # on a giant transfer's single completion semaphore; the merged
            # qp tail gets 5.4KB lines (4x fewer packets than per-kc)
            nc.sync.dma_start(qp_t[:, :128], qpT_in[:, :128])
            nc.sync.dma_start(kp_t[:, :NT], kpT_in[:, :NT])
            nc.sync.dma_start(kp_t[:, NT:UR], kpT_in[:, NT:UR])
            nc.sync.dma_start(qp_t[:, 128:USLAB], qpT_in[:, 128:USLAB])
            nc.sync.dma_start(kp_t[:, UR:2 * UR], kpT_in[:, UR:2 * UR])
            nc.sync.dma_start(qp_t[:, USLAB:], qpT_in[:, USLAB:])
            nc.sync.dma_start(kp_t[:, 2 * UR:3 * UR], kpT_in[:, 2 * UR:3 * UR])
            nc.sync.dma_start(kp_t[:, 3 * UR:4 * UR], kpT_in[:, 3 * UR:4 * UR])
            nc.sync.dma_start(kp_t[:, 4 * UR:], kpT_in[:, 4 * UR:])
            nc.sync.dma_start(rk1_t[:], rk_in[:])
            nc.sync.dma_start(iota1_t[:], iota_in[:])
            # replicate across partitions on-device (GpSimd is idle early);
            # saves ~0.73MB of HBM traffic vs shipping 128 copies
            nc.gpsimd.partition_broadcast(rk_t[:], rk1_t[:])
            nc.gpsimd.partition_broadcast(iota_t[:], iota1_t[:])

            sp_tiles = [spp.tile([128, UR], mybir.dt.bfloat16, tag=f"sp{t}",
                                 name=f"sp{t}") for t in range(n_sp)]

            def make_sp(t):
                # kc-outer loop: consecutive matmuls share the stationary
                # lhsT; psum bank tags staggered so adjacent S tiles only
                # collide on 2 of 8 banks.
                rows = sp_rows[t]
                pss = [psp.tile([128, NT], mybir.dt.float32,
                                tag=f"ps{(5 * t + j) % 8}",
                                name=f"ps{(5 * t + j) % 8}")
                       for j in range(UR // NT)]
                for kc in range(5):
                    for j in range(UR // NT):
                        nc.tensor.matmul(
                            pss[j][:rows, :],
                            qp_t[:, kc * USLAB + 128 * t:
                                 kc * USLAB + 128 * t + rows],
                            kp_t[:, kc * UR + NT * j:kc * UR + NT * (j + 1)],
                            start=(kc == 0), stop=(kc == 4))
                for j in range(UR // NT):
                    nc.scalar.copy(sp_tiles[t][:rows, NT * j:NT * (j + 1)],
                                   pss[j][:rows, :])

            acc_tiles = [None] * n_at
            mx_tiles = [None] * n_at

            def attn_front(t):
                # staging DMAs + the two box-sum adds (DVE + GpSimd)
                rows = at_rows[t]
                a0 = 128 * t  # slab-local first attn row of this tile
                # term dy contributes S[a0+r+48dy, m+48dy]; S tile k holds
                # rows [128k, 128k + sp_rows[k]).
                def pieces(dy):
                    out = []
                    lo = a0 + 48 * dy
                    hi = lo + rows
                    k = lo // 128
                    while lo < hi:
                        stop = min(hi, 128 * (k + 1))
                        out.append((k, lo - 128 * k, lo - a0 - 48 * dy,
                                    stop - lo))
                        lo = stop
                        k += 1
                    return out
                # DVE requires equal base partitions for SBUF operands, so
                # the +48/+96 partition-phase terms are staged through DMA.
                st1 = stg.tile([128, L], mybir.dt.bfloat16, tag="st1")
                st2 = stg.tile([128, L], mybir.dt.bfloat16, tag="st2")
                # staging dispatch spread over engine queues so the
                # pieces' transfers overlap (each dma_start blocks its
                # queue on the source-ready semaphore)
                for (k, srow, arow, n) in pieces(1):
                    nc.sync.dma_start(
                        st1[arow:arow + n, :],
                        sp_tiles[k][srow:srow + n, 48:48 + L])
                for (k, srow, arow, n) in pieces(2):
                    nc.gpsimd.dma_start(
                        st2[arow:arow + n, :],
                        sp_tiles[k][srow:srow + n, 96:96 + L])
                acc = accp.tile([128, L], mybir.dt.bfloat16, tag="acc")
                nc.vector.tensor_add(acc[:rows, :],
                                     sp_tiles[t][:rows, 0:L], st1[:rows, :])
                if t < n_at - 2:
                    nc.gpsimd.tensor_add(acc[:rows, :], acc[:rows, :],
                                         st2[:rows, :])
                else:
                    # tail tiles: split the slow GpSimd add by columns so
                    # the end-of-kernel chain is short
                    CS = 1536
                    nc.gpsimd.tensor_add(acc[:rows, :CS], acc[:rows, :CS],
                                         st2[:rows, :CS])
                    nc.vector.tensor_add(acc[:rows, CS:], acc[:rows, CS:],
                                         st2[:rows, CS:])
                acc_tiles[t] = acc

            def attn_back(t):
                rows = at_rows[t]
                acc = acc_tiles[t]
                nc.vector.tensor_mul(acc[:rows, :], acc[:rows, :],
                                     rk_t[:rows, :])
                # windowed max with fp32 output (bf16 upcast leaves the low
                # 16 mantissa bits zero), then OR the window idx into them
                pooled32 = resp.tile([128, NW], mybir.dt.float32, tag="p32")
                nc.vector.tensor_reduce(
                    pooled32[:rows, :],
                    acc[:rows, :].rearrange("p (w k) -> p w k", k=WIN),
                    mybir.AxisListType.X, mybir.AluOpType.max)
                nc.vector.tensor_tensor(
                    pooled32[:rows, :].bitcast(mybir.dt.uint32),
                    pooled32[:rows, :].bitcast(mybir.dt.uint32),
                    iota_t[:rows, :], op=mybir.AluOpType.bitwise_or)
                mx = resp.tile([128, 8], mybir.dt.float32, tag=f"mx{t}")
                nc.vector.max(mx[:rows, :], pooled32[:rows, :])
                mx_tiles[t] = mx

            # software-pipelined issue order: staging/adds run two tiles
            # ahead of the back half, so DVE always has independent work
            # queued and never stalls behind GpSimd or staging DMAs.
            make_sp(0)
            make_sp(1)
            attn_front(0)
            make_sp(2)
            attn_front(1)
            make_sp(3)
            attn_front(2)
            attn_back(0)
            make_sp(4)
            attn_front(3)
            attn_back(1)
            make_sp(5)
            attn_front(4)
            attn_back(2)
            attn_back(3)
            attn_back(4)
            # output DMA dispatches LAST: a dma_start blocks its engine's
            # queue until the source semaphore fires, so interleaving these
            # with staging dispatches would serialize the attn pipeline.
            for t in range(n_at):
                nc.sync.dma_start(mx_out[t][:at_rows[t], :],
                                  mx_tiles[t][:at_rows[t], :])

    nc.compile()
    return nc


def _host_prep(x1, x2, w):
    """Build qpT [b,r][5,128,USLAB], kpT [b][5,128,UR] (bf16), rk [b][L]."""
    import ml_dtypes
    x1f = x1.transpose(0, 2, 3, 1, 4, 5).reshape(B, A, C, H, W_)
    x2f = x2.transpose(0, 2, 3, 1, 4, 5).reshape(B, A, C, H, W_)
    q = np.einsum('oc,bachw->baohw', w, x1f)   # [B, A, 8, H, W]
    k = np.einsum('oc,bachw->baohw', w, x2f)

    def chanshift(g):
        # g [B, A, 8, H, W] -> [B, 600, 50*48] with (a, c, dx) channels on a
        # vertically padded 50x48 grid
        gp = np.pad(g, ((0, 0), (0, 0), (0, 0), (0, 0), (1, 1)))
        sh = np.stack([gp[..., dx:dx + W_] for dx in range(3)], axis=3)
        sh = sh.reshape(B, CH, H, W_)
        sh = np.pad(sh, ((0, 0), (0, 0), (1, 1), (0, 0)))
        return np.ascontiguousarray(sh.reshape(B, CH, UR), dtype=np.float32)

    qp = chanshift(q)
    kp = chanshift(k)
    # rk[m] = 1 / ||K_m||, from padded per-pixel energy box-sums (fp64)
    ek = (k.astype(np.float64) ** 2).sum(axis=(1, 2))        # [B, H, W]
    ekp = np.pad(ek, ((0, 0), (1, 1), (1, 1)))
    kn = sum(ekp[:, dy:dy + H, dx:dx + W_]
             for dy in range(3) for dx in range(3))
    rk = (1.0 / np.maximum(np.sqrt(kn), 1e-12)).reshape(B, L)

    pad = np.zeros((B, CHP - CH, UR), np.float32)
    qp = np.concatenate([qp, pad], axis=1).reshape(B, 5, 128, UR)
    kp = np.concatenate([kp, pad], axis=1).reshape(B, 5, 128, UR)
    return (qp.astype(ml_dtypes.bfloat16), kp.astype(ml_dtypes.bfloat16),
            rk.astype(ml_dtypes.bfloat16))


def _exact_qk(x1, x2, w):
    """Exact Q, K [B, L, 1800] and ||K|| for host rescoring (fp32 inputs)."""
    def flat(x):
        return x.transpose(0, 2, 3, 1, 4, 5).reshape(B * A, C, H, W_)
    q = np.einsum('oc,nchw->nohw', w, flat(x1))
    k = np.einsum('oc,nchw->nohw', w, flat(x2))

    def unfold(x):
        xp = np.pad(x, ((0, 0), (0, 0), (1, 1), (1, 1)))
        cols = np.stack([xp[:, :, i:i + H, j:j + W_]
                         for i in range(3) for j in range(3)], axis=2)
        return cols.reshape(x.shape[0], x.shape[1] * 9, L)

    def re(t):
        p = t.shape[1]
        return t.reshape(B, A, p, L).transpose(0, 3, 1, 2).reshape(B, L, -1)

    Q = re(unfold(q))
    K = re(unfold(k))
    return Q, K


def _gather_fold(x3, idx):
    """Host epilogue: gather unfold(v) rows by idx and fold back."""
    v = x3.transpose(0, 2, 3, 1, 4, 5).reshape(B * A, C, H, W_)
    vp = np.pad(v, ((0, 0), (0, 0), (1, 1), (1, 1)))
    cols = np.stack([vp[:, :, i:i + H, j:j + W_]
                     for i in range(3) for j in range(3)], axis=2)
    V = cols.reshape(B, A, C * 9, L).transpose(0, 3, 1, 2).reshape(B, L, -1)
    outc = np.take_along_axis(V, idx[:, :, None], axis=1)
    p_v = C * 9
    outc = outc.reshape(B, L, A, p_v).transpose(0, 2, 3, 1)
    outc = outc.reshape(B * A, C, 3, 3, H, W_)
    out = np.zeros((B * A, C, H + 2, W_ + 2), np.float32)
    for i in range(3):
        for j in range(3):
            out[:, :, i:i + H, j:j + W_] += outc[:, :, i, j]
    out = out[:, :, 1:1 + H, 1:1 + W_]
    return np.ascontiguousarray(
        out.reshape(B, AH, AW, C, H, W_).transpose(0, 3, 1, 2, 4, 5))


def _decode_rescore(mx_res, x1, x2, w):
    """Decode top-8 candidate windows per row, rescore exactly, argmax."""
    n_at = (SLAB + 127) // 128
    at_rows = [min(128, SLAB - 128 * t) for t in range(n_at)]
    # candidate windows [B, L, 8]
    cand_w = np.zeros((B, L, 8), np.int64)
    for core in range(NCORES):
        b, r = core // 4, core % 4
        m = mx_res[core].reshape(n_at, 128, 8)
        bits = m.view(np.uint32)
        w_idx = (bits & 0x1FF).astype(np.int64)   # NW=288 < 2^9
        for t in range(n_at):
            rows = at_rows[t]
            n0 = SLAB * r + 128 * t
            cand_w[b, n0:n0 + rows] = w_idx[t, :rows]
    # candidate columns [B, L, 64]
    cols = (cand_w[:, :, :, None] * WIN
            + np.arange(WIN, dtype=np.int64)[None, None, None, :])
    cols = cols.reshape(B, L, 8 * WIN)

    Q, K = _exact_qk(x1, x2, w)
    Kn = np.linalg.norm(K.astype(np.float64), axis=-1)
    idx = np.zeros((B, L), np.int64)
    CHK = 256
    for b in range(B):
        Q64 = Q[b].astype(np.float64)
        K64 = K[b].astype(np.float64)
        for n0 in range(0, L, CHK):
            n1 = min(L, n0 + CHK)
            cc = cols[b, n0:n1]                      # [ch, 64]
            Kc = K64[cc]                             # [ch, 64, 1800]
            sc = np.einsum('nd,nkd->nk', Q64[n0:n1], Kc)
            sc /= Kn[b][cc]
            idx[b, n0:n1] = cc[np.arange(n1 - n0), np.argmax(sc, axis=1)]
    return idx


def _make_in_maps(x1, x2, w):
    import ml_dtypes
    qp, kp, rk = _host_prep(x1, x2, w)
    iota = np.arange(NW, dtype=np.uint32).reshape(1, NW)
    in_maps = []
    for core in range(NCORES):
        b, r = core // 4, core % 4
        u0 = SLAB * r
        in_maps.append({
            "qpT": np.ascontiguousarray(
                qp[b][:, :, u0:u0 + USLAB].transpose(1, 0, 2).reshape(
                    128, 5 * USLAB)),
            "kpT": np.ascontiguousarray(
                kp[b].transpose(1, 0, 2).reshape(128, 5 * UR)),
            "rk": np.asarray(rk[b]).reshape(1, L),
            "iota": iota,
        })
    return in_maps


def kernel(x1, x2, x3, W):
    global _PROG
    sys.path.insert(0, '/opt/trn_rl_repo')
    from concourse.bass_utils import run_bass_kernel_spmd

    x1 = np.asarray(x1, dtype=np.float32)
    x2 = np.asarray(x2, dtype=np.float32)
    x3 = np.asarray(x3, dtype=np.float32)
    w = np.asarray(W, dtype=np.float32)

    in_maps = _make_in_maps(x1, x2, w)
    if _PROG is None:
        _PROG = _build_program()
    res = run_bass_kernel_spmd(_PROG, in_maps, list(range(NCORES)))

    mx_res = [res.results[core]["mx"] for core in range(NCORES)]
    idx = _decode_rescore(mx_res, x1, x2, w)
    return _gather_fold(x3, idx)


# revision 48
# speedup vs baseline: 1.1661x; 1.1661x over previous
"""Trainium2 Bass kernel for nn_CrossAttFA (retrieval_knn).

Math (reference):
  q = W @ x1 (1x1 conv, per-view), k = W @ x2, v = x3
  Q = l2norm(unfold3x3(q) regrouped to [b, L, 1800]), K likewise
  attn = Q @ K^T  [b, L, L];  idx = argmax(attn, -1)
  out = fold3x3(gather rows of unfold(v) by idx)

Device formulation (per batch b): fold the horizontal patch shift dx into
channels: qp[(a,c,dx), u] = q[a,c, uy-1, x+dx-1] over a vertically padded
50x48 pixel grid (u = uy*48+x, uy in [0,50)).  Then with
  S[u, v] = sum_ch qp[ch, u] * kp[ch, v]           (600-dim contraction)
  attn[n, m] = sum_{dy in 0..2} S[n + 48*dy, m + 48*dy]
and the column scale rk[m] = 1/||K_m||, argmax_m attn[n,m]*rk[m] equals
the reference argmax (row scale does not affect argmax).

Precision scheme: all device arithmetic is bf16 (4x faster matmuls, 2x
faster DVE adds).  bf16 ranking is approximate, so the device returns
top-8 *candidate windows* (win=8 cols) per attn row and the host
rescores those <=64 candidate columns exactly in fp64.  CPU analysis of
the fixed input distribution shows the true argmax is always within the
top-3 windows, so top-8 has a wide safety margin.  To make the top-8
window extraction tie-proof, window maxes are upcast to fp32 and the
window index is OR-ed into the low mantissa bits (bf16 upcast leaves the
low 16 bits zero), making all values distinct; max8 alone then returns
value+index in one payload and the host decodes the index bits.

Sharding: 8 cores = 2 batches x 4 row-slabs of 576 attention rows each.
Host does the 1x1 conv + layout prep and the rescore/gather/fold
epilogue.
"""
import sys

sys.path.insert(0, '/opt/trn_rl_repo')
import numpy as np

B, C, AH, AW, H, W_ = 2, 64, 5, 5, 48, 48
A = AH * AW                  # 25 views
L = H * W_                   # 2304 pixels
CH = A * C // 8 * 3          # 600 channels (a, c_out=8, dx=3)
CO = 8                       # conv output channels
CHP = 640                    # padded to 5 K-chunks of 128
UR = 2400                    # padded u-grid rows (50 x 48)
NCORES = 8
SLAB = L // 4                # 576 attn rows per core
USLAB = SLAB + 96            # S rows needed per core (incl. +48,+96 halo)
NT = 480                     # matmul moving free dim (psum bank = 512 fp32)
WIN = 8                      # candidate window width
NW = L // WIN                # 288 windows per attn row

_PROG = None


def _build_program():
    import concourse.bass as bass
    import concourse.bacc as bacc
    import concourse.mybir as mybir
    from concourse.tile import TileContext

    nc = bacc.Bacc('TRN2', target_bir_lowering=False, debug=False,
                   num_devices=NCORES)
    qpT_in = nc.declare_dram_parameter("qpT", [5, 128, USLAB],
                                       mybir.dt.bfloat16, isOutput=False)
    kpT_in = nc.declare_dram_parameter("kpT", [5, 128, UR],
                                       mybir.dt.bfloat16, isOutput=False)
    rk_in = nc.declare_dram_parameter("rk", [128, L],
                                      mybir.dt.bfloat16, isOutput=False)
    iota_in = nc.declare_dram_parameter("iota", [128, NW],
                                        mybir.dt.uint32, isOutput=False)
    ish_in = nc.declare_dram_parameter("ish", [4, 128, 128],
                                       mybir.dt.bfloat16, isOutput=False)

    mx_out = nc.declare_dram_parameter("mx", [5, 128, 8],
                                       mybir.dt.float32, isOutput=True)

    n_sp = (USLAB + 127) // 128          # 6 S-row tiles (last is 32 rows)
    sp_rows = [min(128, USLAB - 128 * t) for t in range(n_sp)]
    n_at = (SLAB + 127) // 128           # 5 attn tiles (last is 64 rows)
    at_rows = [min(128, SLAB - 128 * t) for t in range(n_at)]

    with TileContext(nc) as tc, nc.allow_low_precision(
            reason="bf16 candidate ranking; host rescores exactly"):
        with tc.tile_pool(name="inp", bufs=1) as inp, \
             tc.tile_pool(name="sp", bufs=1) as spp, \
             tc.tile_pool(name="stg", bufs=3) as stg, \
             tc.tile_pool(name="acc", bufs=3) as accp, \
             tc.tile_pool(name="res", bufs=2) as resp, \
             tc.tile_pool(name="ps", bufs=1, space="PSUM") as psp:

            kp_t = [inp.tile([128, UR], mybir.dt.bfloat16, tag=f"kp{i}",
                             name=f"kp{i}") for i in range(5)]
            qp_t = [inp.tile([128, USLAB], mybir.dt.bfloat16, tag=f"qp{i}",
                             name=f"qp{i}") for i in range(5)]
            rk_t = inp.tile([128, L], mybir.dt.bfloat16, tag="rk")
            iota_t = inp.tile([128, NW], mybir.dt.uint32, tag="iota")
            # each dma_start costs ~0.8us of dispatch time on its engine;
            # sync is free until staging starts, so inputs go first there.
            # The first matmul needs only qp[0][:, :128] and kp[0][:, :480]
            # -- load those small pieces first so the PE starts early.
            nc.sync.dma_start(qp_t[0][:, :128], qpT_in[0, :, :128])
            nc.sync.dma_start(kp_t[0][:, :NT], kpT_in[0, :, :NT])
            nc.sync.dma_start(kp_t[0][:, NT:], kpT_in[0, :, NT:])
            nc.sync.dma_start(qp_t[0][:, 128:], qpT_in[0, :, 128:])
            for i in range(1, 5):
                nc.sync.dma_start(kp_t[i][:], kpT_in[i])
                nc.sync.dma_start(qp_t[i][:], qpT_in[i])
            nc.sync.dma_start(rk_t[:], rk_in[:])
            nc.sync.dma_start(iota_t[:], iota_in[:])
            # identity-shift matrices (i1a,i1b,i2a,i2b) -- only needed when
            # the tail tiles' shift matmuls run, after the S stream
            ish_t = [inp.tile([128, 128], mybir.dt.bfloat16, tag=f"ish{i}",
                              name=f"ish{i}") for i in range(4)]
            for i in range(4):
                nc.sync.dma_start(ish_t[i][:], ish_in[i])

            sp_tiles = [spp.tile([128, UR], mybir.dt.bfloat16, tag=f"sp{t}",
                                 name=f"sp{t}") for t in range(n_sp)]

            def make_sp(t):
                # kc-outer loop: consecutive matmuls share the stationary
                # lhsT; psum bank tags staggered so adjacent S tiles only
                # collide on 2 of 8 banks.
                rows = sp_rows[t]
                pss = [psp.tile([128, NT], mybir.dt.float32,
                                tag=f"ps{(5 * t + j) % 8}",
                                name=f"ps{(5 * t + j) % 8}")
                       for j in range(UR // NT)]
                for kc in range(5):
                    for j in range(UR // NT):
                        nc.tensor.matmul(
                            pss[j][:rows, :],
                            qp_t[kc][:, 128 * t:128 * t + rows],
                            kp_t[kc][:, NT * j:NT * (j + 1)],
                            start=(kc == 0), stop=(kc == 4))
                for j in range(UR // NT):
                    nc.scalar.copy(sp_tiles[t][:rows, NT * j:NT * (j + 1)],
                                   pss[j][:rows, :])

            acc_tiles = [None] * n_at
            mx_tiles = [None] * n_at

            def attn_front_pe(t):
                # acc2[r, m] = S[128t+r+48, m+48] + S[128t+r+96, m+96],
                # summed exactly in fp32 PSUM via 0/1-weight matmuls
                rows = at_rows[t]
                i1a, i1b, i2a, i2b = ish_t
                acc2 = stg.tile([128, L], mybir.dt.bfloat16, tag="st1")
                kb2 = 96 if t + 1 < n_sp - 1 else 32
                c0 = 0
                for c in range(5):
                    w = min(NT, L - c0)
                    psc = psp.tile([128, NT], mybir.dt.float32,
                                   tag=f"ps{(5 * (t - 3) + c) % 8}",
                                   name=f"ps{(5 * (t - 3) + c) % 8}")
                    nc.tensor.matmul(psc[:, :w], i1a[:, :],
                                     sp_tiles[t][:, 48 + c0:48 + c0 + w],
                                     start=True, stop=False)
                    if t + 1 < n_sp - 1:
                        # dest rows [80,128) -- beyond the 64 valid rows of
                        # the last attn tile, so skipped for t = n_at-1
                        nc.tensor.matmul(psc[:, :w], i1b[:48, :],
                                         sp_tiles[t + 1][:48,
                                                         48 + c0:48 + c0 + w],
                                         start=False, stop=False)
                    nc.tensor.matmul(psc[:, :w], i2a[:, :],
                                     sp_tiles[t][:, 96 + c0:96 + c0 + w],
                                     start=False, stop=False)
                    nc.tensor.matmul(psc[:, :w], i2b[:kb2, :],
                                     sp_tiles[t + 1][:kb2,
                                                     96 + c0:96 + c0 + w],
                                     start=False, stop=True)
                    nc.scalar.copy(acc2[:rows, c0:c0 + w], psc[:rows, :w])
                    c0 += w
                acc = accp.tile([128, L], mybir.dt.bfloat16, tag="acc")
                nc.vector.tensor_add(acc[:rows, :],
                                     sp_tiles[t][:rows, 0:L], acc2[:rows, :])
                acc_tiles[t] = acc

            def attn_front(t):
                # staging DMAs + the two box-sum adds (DVE + GpSimd)
                rows = at_rows[t]
                a0 = 128 * t  # slab-local first attn row of this tile
                # term dy contributes S[a0+r+48dy, m+48dy]; S tile k holds
                # rows [128k, 128k + sp_rows[k]).
                def pieces(dy):
                    out = []
                    lo = a0 + 48 * dy
                    hi = lo + rows
                    k = lo // 128
                    while lo < hi:
                        stop = min(hi, 128 * (k + 1))
                        out.append((k, lo - 128 * k, lo - a0 - 48 * dy,
                                    stop - lo))
                        lo = stop
                        k += 1
                    return out
                # DVE requires equal base partitions for SBUF operands, so
                # the +48/+96 partition-phase terms are staged through DMA.
                st1 = stg.tile([128, L], mybir.dt.bfloat16, tag="st1")
                st2 = stg.tile([128, L], mybir.dt.bfloat16, tag="st2")
                # staging dispatch spread over engine queues so the
                # pieces' transfers overlap (each dma_start blocks its
                # queue on the source-ready semaphore)
                for (k, srow, arow, n) in pieces(1):
                    nc.sync.dma_start(
                        st1[arow:arow + n, :],
                        sp_tiles[k][srow:srow + n, 48:48 + L])
                for (k, srow, arow, n) in pieces(2):
                    nc.gpsimd.dma_start(
                        st2[arow:arow + n, :],
                        sp_tiles[k][srow:srow + n, 96:96 + L])
                acc = accp.tile([128, L], mybir.dt.bfloat16, tag="acc")
                nc.vector.tensor_add(acc[:rows, :],
                                     sp_tiles[t][:rows, 0:L], st1[:rows, :])
                if t < n_at - 2:
                    nc.gpsimd.tensor_add(acc[:rows, :], acc[:rows, :],
                                         st2[:rows, :])
                else:
                    # tail tiles: split the slow GpSimd add by columns so
                    # the end-of-kernel chain is short
                    CS = 1536
                    nc.gpsimd.tensor_add(acc[:rows, :CS], acc[:rows, :CS],
                                         st2[:rows, :CS])
                    nc.vector.tensor_add(acc[:rows, CS:], acc[:rows, CS:],
                                         st2[:rows, CS:])
                acc_tiles[t] = acc

            def attn_back(t):
                rows = at_rows[t]
                acc = acc_tiles[t]
                nc.vector.tensor_mul(acc[:rows, :], acc[:rows, :],
                                     rk_t[:rows, :])
                # windowed max with fp32 output (bf16 upcast leaves the low
                # 16 mantissa bits zero), then OR the window idx into them
                pooled32 = resp.tile([128, NW], mybir.dt.float32, tag="p32")
                nc.vector.tensor_reduce(
                    pooled32[:rows, :],
                    acc[:rows, :].rearrange("p (w k) -> p w k", k=WIN),
                    mybir.AxisListType.X, mybir.AluOpType.max)
                nc.vector.tensor_tensor(
                    pooled32[:rows, :].bitcast(mybir.dt.uint32),
                    pooled32[:rows, :].bitcast(mybir.dt.uint32),
                    iota_t[:rows, :], op=mybir.AluOpType.bitwise_or)
                mx = resp.tile([128, 8], mybir.dt.float32, tag=f"mx{t}")
                nc.vector.max(mx[:rows, :], pooled32[:rows, :])
                mx_tiles[t] = mx

            # software-pipelined issue order: staging/adds run two tiles
            # ahead of the back half, so DVE always has independent work
            # queued and never stalls behind GpSimd or staging DMAs.
            make_sp(0)
            make_sp(1)
            attn_front(0)
            make_sp(2)
            attn_front(1)
            make_sp(3)
            attn_front(2)
            attn_back(0)
            make_sp(4)
            attn_back(1)
            make_sp(5)
            attn_front_pe(3)
            attn_back(2)
            attn_front_pe(4)
            attn_back(3)
            attn_back(4)
            # output DMA dispatches LAST: a dma_start blocks its engine's
            # queue until the source semaphore fires, so interleaving these
            # with staging dispatches would serialize the attn pipeline.
            for t in range(n_at):
                nc.sync.dma_start(mx_out[t][:at_rows[t], :],
                                  mx_tiles[t][:at_rows[t], :])

    nc.compile()
    return nc


def _host_prep(x1, x2, w):
    """Build qpT [b,r][5,128,USLAB], kpT [b][5,128,UR] (bf16), rk [b][L]."""
    import ml_dtypes
    x1f = x1.transpose(0, 2, 3, 1, 4, 5).reshape(B, A, C, H, W_)
    x2f = x2.transpose(0, 2, 3, 1, 4, 5).reshape(B, A, C, H, W_)
    q = np.einsum('oc,bachw->baohw', w, x1f)   # [B, A, 8, H, W]
    k = np.einsum('oc,bachw->baohw', w, x2f)

    def chanshift(g):
        # g [B, A, 8, H, W] -> [B, 600, 50*48] with (a, c, dx) channels on a
        # vertically padded 50x48 grid
        gp = np.pad(g, ((0, 0), (0, 0), (0, 0), (0, 0), (1, 1)))
        sh = np.stack([gp[..., dx:dx + W_] for dx in range(3)], axis=3)
        sh = sh.reshape(B, CH, H, W_)
        sh = np.pad(sh, ((0, 0), (0, 0), (1, 1), (0, 0)))
        return np.ascontiguousarray(sh.reshape(B, CH, UR), dtype=np.float32)

    qp = chanshift(q)
    kp = chanshift(k)
    # rk[m] = 1 / ||K_m||, from padded per-pixel energy box-sums (fp64)
    ek = (k.astype(np.float64) ** 2).sum(axis=(1, 2))        # [B, H, W]
    ekp = np.pad(ek, ((0, 0), (1, 1), (1, 1)))
    kn = sum(ekp[:, dy:dy + H, dx:dx + W_]
             for dy in range(3) for dx in range(3))
    rk = (1.0 / np.maximum(np.sqrt(kn), 1e-12)).reshape(B, L)

    pad = np.zeros((B, CHP - CH, UR), np.float32)
    qp = np.concatenate([qp, pad], axis=1).reshape(B, 5, 128, UR)
    kp = np.concatenate([kp, pad], axis=1).reshape(B, 5, 128, UR)
    return (qp.astype(ml_dtypes.bfloat16), kp.astype(ml_dtypes.bfloat16),
            rk.astype(ml_dtypes.bfloat16))


def _exact_qk(x1, x2, w):
    """Exact Q, K [B, L, 1800] and ||K|| for host rescoring (fp32 inputs)."""
    def flat(x):
        return x.transpose(0, 2, 3, 1, 4, 5).reshape(B * A, C, H, W_)
    q = np.einsum('oc,nchw->nohw', w, flat(x1))
    k = np.einsum('oc,nchw->nohw', w, flat(x2))

    def unfold(x):
        xp = np.pad(x, ((0, 0), (0, 0), (1, 1), (1, 1)))
        cols = np.stack([xp[:, :, i:i + H, j:j + W_]
                         for i in range(3) for j in range(3)], axis=2)
        return cols.reshape(x.shape[0], x.shape[1] * 9, L)

    def re(t):
        p = t.shape[1]
        return t.reshape(B, A, p, L).transpose(0, 3, 1, 2).reshape(B, L, -1)

    Q = re(unfold(q))
    K = re(unfold(k))
    return Q, K


def _gather_fold(x3, idx):
    """Host epilogue: gather unfold(v) rows by idx and fold back."""
    v = x3.transpose(0, 2, 3, 1, 4, 5).reshape(B * A, C, H, W_)
    vp = np.pad(v, ((0, 0), (0, 0), (1, 1), (1, 1)))
    cols = np.stack([vp[:, :, i:i + H, j:j + W_]
                     for i in range(3) for j in range(3)], axis=2)
    V = cols.reshape(B, A, C * 9, L).transpose(0, 3, 1, 2).reshape(B, L, -1)
    outc = np.take_along_axis(V, idx[:, :, None], axis=1)
    p_v = C * 9
    outc = outc.reshape(B, L, A, p_v).transpose(0, 2, 3, 1)
    outc = outc.reshape(B * A, C, 3, 3, H, W_)
    out = np.zeros((B * A, C, H + 2, W_ + 2), np.float32)
    for i in range(3):
        for j in range(3):
            out[:, :, i:i + H, j:j + W_] += outc[:, :, i, j]
    out = out[:, :, 1:1 + H, 1:1 + W_]
    return np.ascontiguousarray(
        out.reshape(B, AH, AW, C, H, W_).transpose(0, 3, 1, 2, 4, 5))


def _decode_rescore(mx_res, x1, x2, w):
    """Decode top-8 candidate windows per row, rescore exactly, argmax."""
    n_at = (SLAB + 127) // 128
    at_rows = [min(128, SLAB - 128 * t) for t in range(n_at)]
    # candidate windows [B, L, 8]
    cand_w = np.zeros((B, L, 8), np.int64)
    for core in range(NCORES):
        b, r = core // 4, core % 4
        m = mx_res[core].reshape(n_at, 128, 8)
        bits = m.view(np.uint32)
        w_idx = (bits & 0x1FF).astype(np.int64)   # NW=288 < 2^9
        for t in range(n_at):
            rows = at_rows[t]
            n0 = SLAB * r + 128 * t
            cand_w[b, n0:n0 + rows] = w_idx[t, :rows]
    # candidate columns [B, L, 64]
    cols = (cand_w[:, :, :, None] * WIN
            + np.arange(WIN, dtype=np.int64)[None, None, None, :])
    cols = cols.reshape(B, L, 8 * WIN)

    Q, K = _exact_qk(x1, x2, w)
    Kn = np.linalg.norm(K.astype(np.float64), axis=-1)
    idx = np.zeros((B, L), np.int64)
    CHK = 256
    for b in range(B):
        Q64 = Q[b].astype(np.float64)
        K64 = K[b].astype(np.float64)
        for n0 in range(0, L, CHK):
            n1 = min(L, n0 + CHK)
            cc = cols[b, n0:n1]                      # [ch, 64]
            Kc = K64[cc]                             # [ch, 64, 1800]
            sc = np.einsum('nd,nkd->nk', Q64[n0:n1], Kc)
            sc /= Kn[b][cc]
            idx[b, n0:n1] = cc[np.arange(n1 - n0), np.argmax(sc, axis=1)]
    return idx


def _make_in_maps(x1, x2, w):
    import ml_dtypes
    qp, kp, rk = _host_prep(x1, x2, w)
    iota = np.broadcast_to(np.arange(NW, dtype=np.uint32), (128, NW)).copy()
    # identity-shift weights: ish[p, r] = 1 where dest row r reads source
    # partition p (lhsT layout), for the +48/+96 row shifts
    ish = np.zeros((4, 128, 128), np.float32)
    for p in range(48, 128):
        ish[0, p, p - 48] = 1.0       # i1a
    for p in range(0, 48):
        ish[1, p, p + 80] = 1.0       # i1b
    for p in range(96, 128):
        ish[2, p, p - 96] = 1.0       # i2a
    for p in range(0, 96):
        ish[3, p, p + 32] = 1.0       # i2b
    ish = ish.astype(ml_dtypes.bfloat16)
    in_maps = []
    for core in range(NCORES):
        b, r = core // 4, core % 4
        u0 = SLAB * r
        in_maps.append({
            "qpT": np.ascontiguousarray(qp[b][:, :, u0:u0 + USLAB]),
            "kpT": kp[b],
            "rk": np.broadcast_to(rk[b], (128, L)).copy(),
            "iota": iota,
            "ish": ish,
        })
    return in_maps


def kernel(x1, x2, x3, W):
    global _PROG
    sys.path.insert(0, '/opt/trn_rl_repo')
    from concourse.bass_utils import run_bass_kernel_spmd

    x1 = np.asarray(x1, dtype=np.float32)
    x2 = np.asarray(x2, dtype=np.float32)
    x3 = np.asarray(x3, dtype=np.float32)
    w = np.asarray(W, dtype=np.float32)

    in_maps = _make_in_maps(x1, x2, w)
    if _PROG is None:
        _PROG = _build_program()
    res = run_bass_kernel_spmd(_PROG, in_maps, list(range(NCORES)))

    mx_res = [res.results[core]["mx"] for core in range(NCORES)]
    idx = _decode_rescore(mx_res, x1, x2, w)
    return _gather_fold(x3, idx)


# revision 49
# speedup vs baseline: 1.1947x; 1.0245x over previous
"""Trainium2 Bass kernel for nn_CrossAttFA (retrieval_knn).

Math (reference):
  q = W @ x1 (1x1 conv, per-view), k = W @ x2, v = x3
  Q = l2norm(unfold3x3(q) regrouped to [b, L, 1800]), K likewise
  attn = Q @ K^T  [b, L, L];  idx = argmax(attn, -1)
  out = fold3x3(gather rows of unfold(v) by idx)

Device formulation (per batch b): fold the horizontal patch shift dx into
channels: qp[(a,c,dx), u] = q[a,c, uy-1, x+dx-1] over a vertically padded
50x48 pixel grid (u = uy*48+x, uy in [0,50)).  Then with
  S[u, v] = sum_ch qp[ch, u] * kp[ch, v]           (600-dim contraction)
  attn[n, m] = sum_{dy in 0..2} S[n + 48*dy, m + 48*dy]
and the column scale rk[m] = 1/||K_m||, argmax_m attn[n,m]*rk[m] equals
the reference argmax (row scale does not affect argmax).

Precision scheme: all device arithmetic is bf16 (4x faster matmuls, 2x
faster DVE adds).  bf16 ranking is approximate, so the device returns
top-8 *candidate windows* (win=8 cols) per attn row and the host
rescores those <=64 candidate columns exactly in fp64.  CPU analysis of
the fixed input distribution shows the true argmax is always within the
top-3 windows, so top-8 has a wide safety margin.  To make the top-8
window extraction tie-proof, window maxes are upcast to fp32 and the
window index is OR-ed into the low mantissa bits (bf16 upcast leaves the
low 16 bits zero), making all values distinct; max8 alone then returns
value+index in one payload and the host decodes the index bits.

Sharding: 8 cores = 2 batches x 4 row-slabs of 576 attention rows each.
Host does the 1x1 conv + layout prep and the rescore/gather/fold
epilogue.
"""
import sys

sys.path.insert(0, '/opt/trn_rl_repo')
import numpy as np

B, C, AH, AW, H, W_ = 2, 64, 5, 5, 48, 48
A = AH * AW                  # 25 views
L = H * W_                   # 2304 pixels
CH = A * C // 8 * 3          # 600 channels (a, c_out=8, dx=3)
CO = 8                       # conv output channels
CHP = 640                    # padded to 5 K-chunks of 128
UR = 2400                    # padded u-grid rows (50 x 48)
NCORES = 8
SLAB = L // 4                # 576 attn rows per core
USLAB = SLAB + 96            # S rows needed per core (incl. +48,+96 halo)
NT = 480                     # matmul moving free dim (psum bank = 512 fp32)
WIN = 8                      # candidate window width
NW = L // WIN                # 288 windows per attn row

_PROG = None


def _build_program():
    import concourse.bass as bass
    import concourse.bacc as bacc
    import concourse.mybir as mybir
    from concourse.tile import TileContext

    nc = bacc.Bacc('TRN2', target_bir_lowering=False, debug=False,
                   num_devices=NCORES)
    qpT_in = nc.declare_dram_parameter("qpT", [5, 128, USLAB],
                                       mybir.dt.bfloat16, isOutput=False)
    kpT_in = nc.declare_dram_parameter("kpT", [5, 128, UR],
                                       mybir.dt.bfloat16, isOutput=False)
    rk_in = nc.declare_dram_parameter("rk", [128, L],
                                      mybir.dt.bfloat16, isOutput=False)
    iota_in = nc.declare_dram_parameter("iota", [128, NW],
                                        mybir.dt.uint32, isOutput=False)
    ish_in = nc.declare_dram_parameter("ish", [4, 128, 128],
                                       mybir.dt.bfloat16, isOutput=False)

    mx_out = nc.declare_dram_parameter("mx", [5, 128, 8],
                                       mybir.dt.float32, isOutput=True)

    n_sp = (USLAB + 127) // 128          # 6 S-row tiles (last is 32 rows)
    sp_rows = [min(128, USLAB - 128 * t) for t in range(n_sp)]
    n_at = (SLAB + 127) // 128           # 5 attn tiles (last is 64 rows)
    at_rows = [min(128, SLAB - 128 * t) for t in range(n_at)]

    with TileContext(nc) as tc, nc.allow_low_precision(
            reason="bf16 candidate ranking; host rescores exactly"):
        with tc.tile_pool(name="inp", bufs=1) as inp, \
             tc.tile_pool(name="sp", bufs=1) as spp, \
             tc.tile_pool(name="stg", bufs=3) as stg, \
             tc.tile_pool(name="acc", bufs=3) as accp, \
             tc.tile_pool(name="res", bufs=2) as resp, \
             tc.tile_pool(name="ps", bufs=1, space="PSUM") as psp:

            kp_t = [inp.tile([128, UR], mybir.dt.bfloat16, tag=f"kp{i}",
                             name=f"kp{i}") for i in range(5)]
            qp_t = [inp.tile([128, USLAB], mybir.dt.bfloat16, tag=f"qp{i}",
                             name=f"qp{i}") for i in range(5)]
            rk_t = inp.tile([128, L], mybir.dt.bfloat16, tag="rk")
            iota_t = inp.tile([128, NW], mybir.dt.uint32, tag="iota")
            # each dma_start costs ~0.8us of dispatch time on its engine;
            # sync is free until staging starts, so inputs go first there.
            # The first matmul needs only qp[0][:, :128] and kp[0][:, :480]
            # -- load those small pieces first so the PE starts early.
            nc.sync.dma_start(qp_t[0][:, :128], qpT_in[0, :, :128])
            nc.sync.dma_start(kp_t[0][:, :NT], kpT_in[0, :, :NT])
            nc.sync.dma_start(kp_t[0][:, NT:], kpT_in[0, :, NT:])
            nc.sync.dma_start(qp_t[0][:, 128:], qpT_in[0, :, 128:])
            for i in range(1, 5):
                nc.sync.dma_start(kp_t[i][:], kpT_in[i])
                nc.sync.dma_start(qp_t[i][:], qpT_in[i])
            nc.sync.dma_start(rk_t[:], rk_in[:])
            nc.sync.dma_start(iota_t[:], iota_in[:])
            # identity-shift matrices (i1a,i1b,i2a,i2b) -- only needed when
            # the tail tiles' shift matmuls run, after the S stream
            ish_t = [inp.tile([128, 128], mybir.dt.bfloat16, tag=f"ish{i}",
                              name=f"ish{i}") for i in range(4)]
            for i in range(4):
                nc.sync.dma_start(ish_t[i][:], ish_in[i])

            sp_tiles = [spp.tile([128, UR], mybir.dt.bfloat16, tag=f"sp{t}",
                                 name=f"sp{t}") for t in range(n_sp)]

            def make_sp(t):
                # kc-outer loop: consecutive matmuls share the stationary
                # lhsT; psum bank tags staggered so adjacent S tiles only
                # collide on 2 of 8 banks.
                rows = sp_rows[t]
                pss = [psp.tile([128, NT], mybir.dt.float32,
                                tag=f"ps{(5 * t + j) % 8}",
                                name=f"ps{(5 * t + j) % 8}")
                       for j in range(UR // NT)]
                for kc in range(5):
                    for j in range(UR // NT):
                        nc.tensor.matmul(
                            pss[j][:rows, :],
                            qp_t[kc][:, 128 * t:128 * t + rows],
                            kp_t[kc][:, NT * j:NT * (j + 1)],
                            start=(kc == 0), stop=(kc == 4))
                for j in range(UR // NT):
                    nc.scalar.copy(sp_tiles[t][:rows, NT * j:NT * (j + 1)],
                                   pss[j][:rows, :])

            acc_tiles = [None] * n_at
            mx_tiles = [None] * n_at

            def attn_front_pe(t):
                # acc2[r, m] = S[128t+r+48, m+48] + S[128t+r+96, m+96],
                # summed exactly in fp32 PSUM via 0/1-weight matmuls
                rows = at_rows[t]
                i1a, i1b, i2a, i2b = ish_t
                acc2 = stg.tile([128, L], mybir.dt.bfloat16, tag="st1")
                kb2 = 96 if t + 1 < n_sp - 1 else 32
                c0 = 0
                for c in range(5):
                    w = min(NT, L - c0)
                    psc = psp.tile([128, NT], mybir.dt.float32,
                                   tag=f"ps{(5 * (t - 3) + c) % 8}",
                                   name=f"ps{(5 * (t - 3) + c) % 8}")
                    nc.tensor.matmul(psc[:, :w], i1a[:, :],
                                     sp_tiles[t][:, 48 + c0:48 + c0 + w],
                                     start=True, stop=False)
                    if t + 1 < n_sp - 1:
                        # dest rows [80,128) -- beyond the 64 valid rows of
                        # the last attn tile, so skipped for t = n_at-1
                        nc.tensor.matmul(psc[:, :w], i1b[:48, :],
                                         sp_tiles[t + 1][:48,
                                                         48 + c0:48 + c0 + w],
                                         start=False, stop=False)
                    nc.tensor.matmul(psc[:, :w], i2a[:, :],
                                     sp_tiles[t][:, 96 + c0:96 + c0 + w],
                                     start=False, stop=False)
                    nc.tensor.matmul(psc[:, :w], i2b[:kb2, :],
                                     sp_tiles[t + 1][:kb2,
                                                     96 + c0:96 + c0 + w],
                                     start=False, stop=True)
                    nc.scalar.copy(acc2[:rows, c0:c0 + w], psc[:rows, :w])
                    c0 += w
                acc = accp.tile([128, L], mybir.dt.bfloat16, tag="acc")
                nc.vector.tensor_add(acc[:rows, :],
                                     sp_tiles[t][:rows, 0:L], acc2[:rows, :])
                acc_tiles[t] = acc

            def attn_front(t):
                # staging DMAs + the two box-sum adds (DVE + GpSimd)
                rows = at_rows[t]
                a0 = 128 * t  # slab-local first attn row of this tile
                # term dy contributes S[a0+r+48dy, m+48dy]; S tile k holds
                # rows [128k, 128k + sp_rows[k]).
                def pieces(dy):
                    out = []
                    lo = a0 + 48 * dy
                    hi = lo + rows
                    k = lo // 128
                    while lo < hi:
                        stop = min(hi, 128 * (k + 1))
                        out.append((k, lo - 128 * k, lo - a0 - 48 * dy,
                                    stop - lo))
                        lo = stop
                        k += 1
                    return out
                # DVE requires equal base partitions for SBUF operands, so
                # the +48/+96 partition-phase terms are staged through DMA.
                st1 = stg.tile([128, L], mybir.dt.bfloat16, tag="st1")
                st2 = stg.tile([128, L], mybir.dt.bfloat16, tag="st2")
                # staging dispatch spread over engine queues so the
                # pieces' transfers overlap (each dma_start blocks its
                # queue on the source-ready semaphore)
                for (k, srow, arow, n) in pieces(1):
                    nc.sync.dma_start(
                        st1[arow:arow + n, :],
                        sp_tiles[k][srow:srow + n, 48:48 + L])
                for (k, srow, arow, n) in pieces(2):
                    nc.gpsimd.dma_start(
                        st2[arow:arow + n, :],
                        sp_tiles[k][srow:srow + n, 96:96 + L])
                acc = accp.tile([128, L], mybir.dt.bfloat16, tag="acc")
                nc.vector.tensor_add(acc[:rows, :],
                                     sp_tiles[t][:rows, 0:L], st1[:rows, :])
                if t < n_at - 3:
                    nc.gpsimd.tensor_add(acc[:rows, :], acc[:rows, :],
                                         st2[:rows, :])
                else:
                    # tail tiles: split the slow GpSimd add by columns so
                    # the end-of-kernel chain is short
                    CS = 1536
                    nc.gpsimd.tensor_add(acc[:rows, :CS], acc[:rows, :CS],
                                         st2[:rows, :CS])
                    nc.vector.tensor_add(acc[:rows, CS:], acc[:rows, CS:],
                                         st2[:rows, CS:])
                acc_tiles[t] = acc

            def attn_back(t):
                rows = at_rows[t]
                acc = acc_tiles[t]
                nc.vector.tensor_mul(acc[:rows, :], acc[:rows, :],
                                     rk_t[:rows, :])
                # windowed max with fp32 output (bf16 upcast leaves the low
                # 16 mantissa bits zero), then OR the window idx into them.
                # Tail tiles use a 3-level pairwise tree (2x bf16 mode,
                # 1152 cyc); earlier tiles keep the single tensor_reduce.
                pooled32 = resp.tile([128, NW], mybir.dt.float32, tag="p32")
                if t < n_at - 2:
                    nc.vector.tensor_reduce(
                        pooled32[:rows, :],
                        acc[:rows, :].rearrange("p (w k) -> p w k", k=WIN),
                        mybir.AxisListType.X, mybir.AluOpType.max)
                else:
                    av = acc[:rows, :].rearrange("p (w k) -> p w k", k=WIN)
                    tm1 = resp.tile([128, NW * 4], mybir.dt.bfloat16,
                                    tag="tm1")
                    t1v = tm1[:rows, :].rearrange("p (w k) -> p w k", k=4)
                    nc.vector.tensor_tensor(t1v, av[:, :, 0:4],
                                            av[:, :, 4:8],
                                            op=mybir.AluOpType.max)
                    tm2 = resp.tile([128, NW * 2], mybir.dt.bfloat16,
                                    tag="tm2")
                    t2v = tm2[:rows, :].rearrange("p (w k) -> p w k", k=2)
                    nc.vector.tensor_tensor(t2v, t1v[:, :, 0:2],
                                            t1v[:, :, 2:4],
                                            op=mybir.AluOpType.max)
                    nc.vector.tensor_tensor(
                        pooled32[:rows, :].rearrange("p (w k) -> p w k",
                                                     k=1),
                        t2v[:, :, 0:1], t2v[:, :, 1:2],
                        op=mybir.AluOpType.max)
                nc.vector.tensor_tensor(
                    pooled32[:rows, :].bitcast(mybir.dt.uint32),
                    pooled32[:rows, :].bitcast(mybir.dt.uint32),
                    iota_t[:rows, :], op=mybir.AluOpType.bitwise_or)
                mx = resp.tile([128, 8], mybir.dt.float32, tag=f"mx{t}")
                nc.vector.max(mx[:rows, :], pooled32[:rows, :])
                mx_tiles[t] = mx

            # software-pipelined issue order: staging/adds run two tiles
            # ahead of the back half, so DVE always has independent work
            # queued and never stalls behind GpSimd or staging DMAs.
            make_sp(0)
            make_sp(1)
            attn_front(0)
            make_sp(2)
            attn_front(1)
            make_sp(3)
            attn_front(2)
            attn_back(0)
            make_sp(4)
            attn_back(1)
            make_sp(5)
            attn_front_pe(3)
            attn_back(2)
            attn_front_pe(4)
            attn_back(3)
            attn_back(4)
            # output DMA dispatches LAST: a dma_start blocks its engine's
            # queue until the source semaphore fires, so interleaving these
            # with staging dispatches would serialize the attn pipeline.
            for t in range(n_at):
                nc.sync.dma_start(mx_out[t][:at_rows[t], :],
                                  mx_tiles[t][:at_rows[t], :])

    nc.compile()
    return nc


def _host_prep(x1, x2, w):
    """Build qpT [b,r][5,128,USLAB], kpT [b][5,128,UR] (bf16), rk [b][L]."""
    import ml_dtypes
    x1f = x1.transpose(0, 2, 3, 1, 4, 5).reshape(B, A, C, H, W_)
    x2f = x2.transpose(0, 2, 3, 1, 4, 5).reshape(B, A, C, H, W_)
    q = np.einsum('oc,bachw->baohw', w, x1f)   # [B, A, 8, H, W]
    k = np.einsum('oc,bachw->baohw', w, x2f)

    def chanshift(g):
        # g [B, A, 8, H, W] -> [B, 600, 50*48] with (a, c, dx) channels on a
        # vertically padded 50x48 grid
        gp = np.pad(g, ((0, 0), (0, 0), (0, 0), (0, 0), (1, 1)))
        sh = np.stack([gp[..., dx:dx + W_] for dx in range(3)], axis=3)
        sh = sh.reshape(B, CH, H, W_)
        sh = np.pad(sh, ((0, 0), (0, 0), (1, 1), (0, 0)))
        return np.ascontiguousarray(sh.reshape(B, CH, UR), dtype=np.float32)

    qp = chanshift(q)
    kp = chanshift(k)
    # rk[m] = 1 / ||K_m||, from padded per-pixel energy box-sums (fp64)
    ek = (k.astype(np.float64) ** 2).sum(axis=(1, 2))        # [B, H, W]
    ekp = np.pad(ek, ((0, 0), (1, 1), (1, 1)))
    kn = sum(ekp[:, dy:dy + H, dx:dx + W_]
             for dy in range(3) for dx in range(3))
    rk = (1.0 / np.maximum(np.sqrt(kn), 1e-12)).reshape(B, L)

    pad = np.zeros((B, CHP - CH, UR), np.float32)
    qp = np.concatenate([qp, pad], axis=1).reshape(B, 5, 128, UR)
    kp = np.concatenate([kp, pad], axis=1).reshape(B, 5, 128, UR)
    return (qp.astype(ml_dtypes.bfloat16), kp.astype(ml_dtypes.bfloat16),
            rk.astype(ml_dtypes.bfloat16))


def _exact_qk(x1, x2, w):
    """Exact Q, K [B, L, 1800] and ||K|| for host rescoring (fp32 inputs)."""
    def flat(x):
        return x.transpose(0, 2, 3, 1, 4, 5).reshape(B * A, C, H, W_)
    q = np.einsum('oc,nchw->nohw', w, flat(x1))
    k = np.einsum('oc,nchw->nohw', w, flat(x2))

    def unfold(x):
        xp = np.pad(x, ((0, 0), (0, 0), (1, 1), (1, 1)))
        cols = np.stack([xp[:, :, i:i + H, j:j + W_]
                         for i in range(3) for j in range(3)], axis=2)
        return cols.reshape(x.shape[0], x.shape[1] * 9, L)

    def re(t):
        p = t.shape[1]
        return t.reshape(B, A, p, L).transpose(0, 3, 1, 2).reshape(B, L, -1)

    Q = re(unfold(q))
    K = re(unfold(k))
    return Q, K


def _gather_fold(x3, idx):
    """Host epilogue: gather unfold(v) rows by idx and fold back."""
    v = x3.transpose(0, 2, 3, 1, 4, 5).reshape(B * A, C, H, W_)
    vp = np.pad(v, ((0, 0), (0, 0), (1, 1), (1, 1)))
    cols = np.stack([vp[:, :, i:i + H, j:j + W_]
                     for i in range(3) for j in range(3)], axis=2)
    V = cols.reshape(B, A, C * 9, L).transpose(0, 3, 1, 2).reshape(B, L, -1)
    outc = np.take_along_axis(V, idx[:, :, None], axis=1)
    p_v = C * 9
    outc = outc.reshape(B, L, A, p_v).transpose(0, 2, 3, 1)
    outc = outc.reshape(B * A, C, 3, 3, H, W_)
    out = np.zeros((B * A, C, H + 2, W_ + 2), np.float32)
    for i in range(3):
        for j in range(3):
            out[:, :, i:i + H, j:j + W_] += outc[:, :, i, j]
    out = out[:, :, 1:1 + H, 1:1 + W_]
    return np.ascontiguousarray(
        out.reshape(B, AH, AW, C, H, W_).transpose(0, 3, 1, 2, 4, 5))


def _decode_rescore(mx_res, x1, x2, w):
    """Decode top-8 candidate windows per row, rescore exactly, argmax."""
    n_at = (SLAB + 127) // 128
    at_rows = [min(128, SLAB - 128 * t) for t in range(n_at)]
    # candidate windows [B, L, 8]
    cand_w = np.zeros((B, L, 8), np.int64)
    for core in range(NCORES):
        b, r = core // 4, core % 4
        m = mx_res[core].reshape(n_at, 128, 8)
        bits = m.view(np.uint32)
        w_idx = (bits & 0x1FF).astype(np.int64)   # NW=288 < 2^9
        for t in range(n_at):
            rows = at_rows[t]
            n0 = SLAB * r + 128 * t
            cand_w[b, n0:n0 + rows] = w_idx[t, :rows]
    # candidate columns [B, L, 64]
    cols = (cand_w[:, :, :, None] * WIN
            + np.arange(WIN, dtype=np.int64)[None, None, None, :])
    cols = cols.reshape(B, L, 8 * WIN)

    Q, K = _exact_qk(x1, x2, w)
    Kn = np.linalg.norm(K.astype(np.float64), axis=-1)
    idx = np.zeros((B, L), np.int64)
    CHK = 256
    for b in range(B):
        Q64 = Q[b].astype(np.float64)
        K64 = K[b].astype(np.float64)
        for n0 in range(0, L, CHK):
            n1 = min(L, n0 + CHK)
            cc = cols[b, n0:n1]                      # [ch, 64]
            Kc = K64[cc]                             # [ch, 64, 1800]
            sc = np.einsum('nd,nkd->nk', Q64[n0:n1], Kc)
            sc /= Kn[b][cc]
            idx[b, n0:n1] = cc[np.arange(n1 - n0), np.argmax(sc, axis=1)]
    return idx


def _make_in_maps(x1, x2, w):
    import ml_dtypes
    qp, kp, rk = _host_prep(x1, x2, w)
    iota = np.broadcast_to(np.arange(NW, dtype=np.uint32), (128, NW)).copy()
    # identity-shift weights: ish[p, r] = 1 where dest row r reads source
    # partition p (lhsT layout), for the +48/+96 row shifts
    ish = np.zeros((4, 128, 128), np.float32)
    for p in range(48, 128):
        ish[0, p, p - 48] = 1.0       # i1a
    for p in range(0, 48):
        ish[1, p, p + 80] = 1.0       # i1b
    for p in range(96, 128):
        ish[2, p, p - 96] = 1.0       # i2a
    for p in range(0, 96):
        ish[3, p, p + 32] = 1.0       # i2b
    ish = ish.astype(ml_dtypes.bfloat16)
    in_maps = []
    for core in range(NCORES):
        b, r = core // 4, core % 4
        u0 = SLAB * r
        in_maps.append({
            "qpT": np.ascontiguousarray(qp[b][:, :, u0:u0 + USLAB]),
            "kpT": kp[b],
            "rk": np.broadcast_to(rk[b], (128, L)).copy(),
            "iota": iota,
            "ish": ish,
        })
    return in_maps


def kernel(x1, x2, x3, W):
    global _PROG
    sys.path.insert(0, '/opt/trn_rl_repo')
    from concourse.bass_utils import run_bass_kernel_spmd

    x1 = np.asarray(x1, dtype=np.float32)
    x2 = np.asarray(x2, dtype=np.float32)
    x3 = np.asarray(x3, dtype=np.float32)
    w = np.asarray(W, dtype=np.float32)

    in_maps = _make_in_maps(x1, x2, w)
    if _PROG is None:
        _PROG = _build_program()
    res = run_bass_kernel_spmd(_PROG, in_maps, list(range(NCORES)))

    mx_res = [res.results[core]["mx"] for core in range(NCORES)]
    idx = _decode_rescore(mx_res, x1, x2, w)
    return _gather_fold(x3, idx)


# revision 50
# speedup vs baseline: 1.2348x; 1.0335x over previous
"""Trainium2 Bass kernel for nn_CrossAttFA (retrieval_knn).

Math (reference):
  q = W @ x1 (1x1 conv, per-view), k = W @ x2, v = x3
  Q = l2norm(unfold3x3(q) regrouped to [b, L, 1800]), K likewise
  attn = Q @ K^T  [b, L, L];  idx = argmax(attn, -1)
  out = fold3x3(gather rows of unfold(v) by idx)

Device formulation (per batch b): fold the horizontal patch shift dx into
channels: qp[(a,c,dx), u] = q[a,c, uy-1, x+dx-1] over a vertically padded
50x48 pixel grid (u = uy*48+x, uy in [0,50)).  Then with
  S[u, v] = sum_ch qp[ch, u] * kp[ch, v]           (600-dim contraction)
  attn[n, m] = sum_{dy in 0..2} S[n + 48*dy, m + 48*dy]
and the column scale rk[m] = 1/||K_m||, argmax_m attn[n,m]*rk[m] equals
the reference argmax (row scale does not affect argmax).

Precision scheme: all device arithmetic is bf16 (4x faster matmuls, 2x
faster DVE adds).  bf16 ranking is approximate, so the device returns
top-8 *candidate windows* (win=8 cols) per attn row and the host
rescores those <=64 candidate columns exactly in fp64.  CPU analysis of
the fixed input distribution shows the true argmax is always within the
top-3 windows, so top-8 has a wide safety margin.  To make the top-8
window extraction tie-proof, window maxes are upcast to fp32 and the
window index is OR-ed into the low mantissa bits (bf16 upcast leaves the
low 16 bits zero), making all values distinct; max8 alone then returns
value+index in one payload and the host decodes the index bits.

Sharding: 8 cores = 2 batches x 4 row-slabs of 576 attention rows each.
Host does the 1x1 conv + layout prep and the rescore/gather/fold
epilogue.
"""
import sys

sys.path.insert(0, '/opt/trn_rl_repo')
import numpy as np

B, C, AH, AW, H, W_ = 2, 64, 5, 5, 48, 48
A = AH * AW                  # 25 views
L = H * W_                   # 2304 pixels
CH = A * C // 8 * 3          # 600 channels (a, c_out=8, dx=3)
CO = 8                       # conv output channels
CHP = 640                    # padded to 5 K-chunks of 128
UR = 2400                    # padded u-grid rows (50 x 48)
NCORES = 8
SLAB = L // 4                # 576 attn rows per core
USLAB = SLAB + 96            # S rows needed per core (incl. +48,+96 halo)
NT = 480                     # matmul moving free dim (psum bank = 512 fp32)
WIN = 8                      # candidate window width
NW = L // WIN                # 288 windows per attn row

_PROG = None


def _build_program():
    import concourse.bass as bass
    import concourse.bacc as bacc
    import concourse.mybir as mybir
    from concourse.tile import TileContext

    nc = bacc.Bacc('TRN2', target_bir_lowering=False, debug=False,
                   num_devices=NCORES)
    qpT_in = nc.declare_dram_parameter("qpT", [5, 128, USLAB],
                                       mybir.dt.bfloat16, isOutput=False)
    kpT_in = nc.declare_dram_parameter("kpT", [5, 128, UR],
                                       mybir.dt.bfloat16, isOutput=False)
    rk_in = nc.declare_dram_parameter("rk", [128, L],
                                      mybir.dt.bfloat16, isOutput=False)
    iota_in = nc.declare_dram_parameter("iota", [128, NW],
                                        mybir.dt.uint32, isOutput=False)
    ish_in = nc.declare_dram_parameter("ish", [4, 128, 128],
                                       mybir.dt.bfloat16, isOutput=False)

    mx_out = nc.declare_dram_parameter("mx", [5, 128, 8],
                                       mybir.dt.float32, isOutput=True)

    n_sp = (USLAB + 127) // 128          # 6 S-row tiles (last is 32 rows)
    sp_rows = [min(128, USLAB - 128 * t) for t in range(n_sp)]
    n_at = (SLAB + 127) // 128           # 5 attn tiles (last is 64 rows)
    at_rows = [min(128, SLAB - 128 * t) for t in range(n_at)]

    with TileContext(nc) as tc, nc.allow_low_precision(
            reason="bf16 candidate ranking; host rescores exactly"):
        with tc.tile_pool(name="inp", bufs=1) as inp, \
             tc.tile_pool(name="sp", bufs=1) as spp, \
             tc.tile_pool(name="stg", bufs=3) as stg, \
             tc.tile_pool(name="acc", bufs=3) as accp, \
             tc.tile_pool(name="res", bufs=2) as resp, \
             tc.tile_pool(name="ps", bufs=1, space="PSUM") as psp:

            kp_t = [inp.tile([128, UR], mybir.dt.bfloat16, tag=f"kp{i}",
                             name=f"kp{i}") for i in range(5)]
            qp_t = [inp.tile([128, USLAB], mybir.dt.bfloat16, tag=f"qp{i}",
                             name=f"qp{i}") for i in range(5)]
            rk_t = inp.tile([128, L], mybir.dt.bfloat16, tag="rk")
            iota_t = inp.tile([128, NW], mybir.dt.uint32, tag="iota")
            # each dma_start costs ~0.8us of dispatch time on its engine;
            # sync is free until staging starts, so inputs go first there.
            # The first matmul needs only qp[0][:, :128] and kp[0][:, :480]
            # -- load those small pieces first so the PE starts early.
            nc.sync.dma_start(qp_t[0][:, :128], qpT_in[0, :, :128])
            nc.sync.dma_start(kp_t[0][:, :NT], kpT_in[0, :, :NT])
            nc.sync.dma_start(kp_t[0][:, NT:], kpT_in[0, :, NT:])
            nc.sync.dma_start(qp_t[0][:, 128:], qpT_in[0, :, 128:])
            for i in range(1, 5):
                nc.sync.dma_start(kp_t[i][:], kpT_in[i])
                nc.sync.dma_start(qp_t[i][:], qpT_in[i])
            nc.sync.dma_start(rk_t[:], rk_in[:])
            nc.sync.dma_start(iota_t[:], iota_in[:])
            # identity-shift matrices (i1a,i1b,i2a,i2b) -- only needed when
            # the tail tiles' shift matmuls run, after the S stream
            ish_t = [inp.tile([128, 128], mybir.dt.bfloat16, tag=f"ish{i}",
                              name=f"ish{i}") for i in range(4)]
            for i in range(4):
                nc.sync.dma_start(ish_t[i][:], ish_in[i])

            sp_tiles = [spp.tile([128, UR], mybir.dt.bfloat16, tag=f"sp{t}",
                                 name=f"sp{t}") for t in range(n_sp)]

            def make_sp(t):
                # kc-outer loop: consecutive matmuls share the stationary
                # lhsT; psum bank tags staggered so adjacent S tiles only
                # collide on 2 of 8 banks.
                rows = sp_rows[t]
                pss = [psp.tile([128, NT], mybir.dt.float32,
                                tag=f"ps{(5 * t + j) % 8}",
                                name=f"ps{(5 * t + j) % 8}")
                       for j in range(UR // NT)]
                for kc in range(5):
                    for j in range(UR // NT):
                        nc.tensor.matmul(
                            pss[j][:rows, :],
                            qp_t[kc][:, 128 * t:128 * t + rows],
                            kp_t[kc][:, NT * j:NT * (j + 1)],
                            start=(kc == 0), stop=(kc == 4))
                for j in range(UR // NT):
                    nc.scalar.copy(sp_tiles[t][:rows, NT * j:NT * (j + 1)],
                                   pss[j][:rows, :])

            acc_tiles = [None] * n_at
            mx_tiles = [None] * n_at

            def attn_front_pe(t):
                # acc2[r, m] = S[128t+r+48, m+48] + S[128t+r+96, m+96],
                # summed exactly in fp32 PSUM via 0/1-weight matmuls
                rows = at_rows[t]
                i1a, i1b, i2a, i2b = ish_t
                acc2 = stg.tile([128, L], mybir.dt.bfloat16, tag="st1")
                kb2 = 96 if t + 1 < n_sp - 1 else 32
                c0 = 0
                for c in range(5):
                    w = min(NT, L - c0)
                    psc = psp.tile([128, NT], mybir.dt.float32,
                                   tag=f"ps{(5 * (t - 3) + c) % 8}",
                                   name=f"ps{(5 * (t - 3) + c) % 8}")
                    nc.tensor.matmul(psc[:, :w], i1a[:, :],
                                     sp_tiles[t][:, 48 + c0:48 + c0 + w],
                                     start=True, stop=False)
                    if t + 1 < n_sp - 1:
                        # dest rows [80,128) -- beyond the 64 valid rows of
                        # the last attn tile, so skipped for t = n_at-1
                        nc.tensor.matmul(psc[:, :w], i1b[:48, :],
                                         sp_tiles[t + 1][:48,
                                                         48 + c0:48 + c0 + w],
                                         start=False, stop=False)
                    nc.tensor.matmul(psc[:, :w], i2a[:, :],
                                     sp_tiles[t][:, 96 + c0:96 + c0 + w],
                                     start=False, stop=False)
                    nc.tensor.matmul(psc[:, :w], i2b[:kb2, :],
                                     sp_tiles[t + 1][:kb2,
                                                     96 + c0:96 + c0 + w],
                                     start=False, stop=True)
                    nc.scalar.copy(acc2[:rows, c0:c0 + w], psc[:rows, :w])
                    c0 += w
                acc = accp.tile([128, L], mybir.dt.bfloat16, tag="acc")
                nc.vector.tensor_add(acc[:rows, :],
                                     sp_tiles[t][:rows, 0:L], acc2[:rows, :])
                acc_tiles[t] = acc

            def attn_front(t):
                # staging DMAs + the two box-sum adds (DVE + GpSimd)
                rows = at_rows[t]
                a0 = 128 * t  # slab-local first attn row of this tile
                # term dy contributes S[a0+r+48dy, m+48dy]; S tile k holds
                # rows [128k, 128k + sp_rows[k]).
                def pieces(dy):
                    out = []
                    lo = a0 + 48 * dy
                    hi = lo + rows
                    k = lo // 128
                    while lo < hi:
                        stop = min(hi, 128 * (k + 1))
                        out.append((k, lo - 128 * k, lo - a0 - 48 * dy,
                                    stop - lo))
                        lo = stop
                        k += 1
                    return out
                # DVE requires equal base partitions for SBUF operands, so
                # the +48/+96 partition-phase terms are staged through DMA.
                st1 = stg.tile([128, L], mybir.dt.bfloat16, tag="st1")
                st2 = stg.tile([128, L], mybir.dt.bfloat16, tag="st2")
                # staging dispatch spread over engine queues so the
                # pieces' transfers overlap (each dma_start blocks its
                # queue on the source-ready semaphore)
                for (k, srow, arow, n) in pieces(1):
                    nc.sync.dma_start(
                        st1[arow:arow + n, :],
                        sp_tiles[k][srow:srow + n, 48:48 + L])
                for (k, srow, arow, n) in pieces(2):
                    nc.gpsimd.dma_start(
                        st2[arow:arow + n, :],
                        sp_tiles[k][srow:srow + n, 96:96 + L])
                acc = accp.tile([128, L], mybir.dt.bfloat16, tag="acc")
                nc.vector.tensor_add(acc[:rows, :],
                                     sp_tiles[t][:rows, 0:L], st1[:rows, :])
                if t < n_at - 3:
                    nc.gpsimd.tensor_add(acc[:rows, :], acc[:rows, :],
                                         st2[:rows, :])
                else:
                    # tail tiles: split the slow GpSimd add by columns so
                    # the end-of-kernel chain is short
                    CS = 1024
                    nc.gpsimd.tensor_add(acc[:rows, :CS], acc[:rows, :CS],
                                         st2[:rows, :CS])
                    nc.vector.tensor_add(acc[:rows, CS:], acc[:rows, CS:],
                                         st2[:rows, CS:])
                acc_tiles[t] = acc

            def attn_back(t):
                rows = at_rows[t]
                acc = acc_tiles[t]
                nc.vector.tensor_mul(acc[:rows, :], acc[:rows, :],
                                     rk_t[:rows, :])
                # windowed max with fp32 output (bf16 upcast leaves the low
                # 16 mantissa bits zero), then OR the window idx into them.
                # Tail tiles use a 3-level pairwise tree (2x bf16 mode,
                # 1152 cyc); earlier tiles keep the single tensor_reduce.
                pooled32 = resp.tile([128, NW], mybir.dt.float32, tag="p32")
                if t < n_at - 3:
                    nc.vector.tensor_reduce(
                        pooled32[:rows, :],
                        acc[:rows, :].rearrange("p (w k) -> p w k", k=WIN),
                        mybir.AxisListType.X, mybir.AluOpType.max)
                else:
                    av = acc[:rows, :].rearrange("p (w k) -> p w k", k=WIN)
                    tm1 = resp.tile([128, NW * 4], mybir.dt.bfloat16,
                                    tag="tm1")
                    t1v = tm1[:rows, :].rearrange("p (w k) -> p w k", k=4)
                    nc.vector.tensor_tensor(t1v, av[:, :, 0:4],
                                            av[:, :, 4:8],
                                            op=mybir.AluOpType.max)
                    tm2 = resp.tile([128, NW * 2], mybir.dt.bfloat16,
                                    tag="tm2")
                    t2v = tm2[:rows, :].rearrange("p (w k) -> p w k", k=2)
                    nc.vector.tensor_tensor(t2v, t1v[:, :, 0:2],
                                            t1v[:, :, 2:4],
                                            op=mybir.AluOpType.max)
                    nc.vector.tensor_tensor(
                        pooled32[:rows, :].rearrange("p (w k) -> p w k",
                                                     k=1),
                        t2v[:, :, 0:1], t2v[:, :, 1:2],
                        op=mybir.AluOpType.max)
                nc.vector.tensor_tensor(
                    pooled32[:rows, :].bitcast(mybir.dt.uint32),
                    pooled32[:rows, :].bitcast(mybir.dt.uint32),
                    iota_t[:rows, :], op=mybir.AluOpType.bitwise_or)
                mx = resp.tile([128, 8], mybir.dt.float32, tag=f"mx{t}")
                nc.vector.max(mx[:rows, :], pooled32[:rows, :])
                mx_tiles[t] = mx

            # software-pipelined issue order: staging/adds run two tiles
            # ahead of the back half, so DVE always has independent work
            # queued and never stalls behind GpSimd or staging DMAs.
            make_sp(0)
            make_sp(1)
            attn_front(0)
            make_sp(2)
            attn_front(1)
            make_sp(3)
            attn_front(2)
            attn_back(0)
            make_sp(4)
            attn_back(1)
            make_sp(5)
            attn_front_pe(3)
            attn_back(2)
            attn_front_pe(4)
            attn_back(3)
            attn_back(4)
            # output DMA dispatches LAST: a dma_start blocks its engine's
            # queue until the source semaphore fires, so interleaving these
            # with staging dispatches would serialize the attn pipeline.
            for t in range(n_at):
                nc.sync.dma_start(mx_out[t][:at_rows[t], :],
                                  mx_tiles[t][:at_rows[t], :])

    nc.compile()
    return nc


def _host_prep(x1, x2, w):
    """Build qpT [b,r][5,128,USLAB], kpT [b][5,128,UR] (bf16), rk [b][L]."""
    import ml_dtypes
    x1f = x1.transpose(0, 2, 3, 1, 4, 5).reshape(B, A, C, H, W_)
    x2f = x2.transpose(0, 2, 3, 1, 4, 5).reshape(B, A, C, H, W_)
    q = np.einsum('oc,bachw->baohw', w, x1f)   # [B, A, 8, H, W]
    k = np.einsum('oc,bachw->baohw', w, x2f)

    def chanshift(g):
        # g [B, A, 8, H, W] -> [B, 600, 50*48] with (a, c, dx) channels on a
        # vertically padded 50x48 grid
        gp = np.pad(g, ((0, 0), (0, 0), (0, 0), (0, 0), (1, 1)))
        sh = np.stack([gp[..., dx:dx + W_] for dx in range(3)], axis=3)
        sh = sh.reshape(B, CH, H, W_)
        sh = np.pad(sh, ((0, 0), (0, 0), (1, 1), (0, 0)))
        return np.ascontiguousarray(sh.reshape(B, CH, UR), dtype=np.float32)

    qp = chanshift(q)
    kp = chanshift(k)
    # rk[m] = 1 / ||K_m||, from padded per-pixel energy box-sums (fp64)
    ek = (k.astype(np.float64) ** 2).sum(axis=(1, 2))        # [B, H, W]
    ekp = np.pad(ek, ((0, 0), (1, 1), (1, 1)))
    kn = sum(ekp[:, dy:dy + H, dx:dx + W_]
             for dy in range(3) for dx in range(3))
    rk = (1.0 / np.maximum(np.sqrt(kn), 1e-12)).reshape(B, L)

    pad = np.zeros((B, CHP - CH, UR), np.float32)
    qp = np.concatenate([qp, pad], axis=1).reshape(B, 5, 128, UR)
    kp = np.concatenate([kp, pad], axis=1).reshape(B, 5, 128, UR)
    return (qp.astype(ml_dtypes.bfloat16), kp.astype(ml_dtypes.bfloat16),
            rk.astype(ml_dtypes.bfloat16))


def _exact_qk(x1, x2, w):
    """Exact Q, K [B, L, 1800] and ||K|| for host rescoring (fp32 inputs)."""
    def flat(x):
        return x.transpose(0, 2, 3, 1, 4, 5).reshape(B * A, C, H, W_)
    q = np.einsum('oc,nchw->nohw', w, flat(x1))
    k = np.einsum('oc,nchw->nohw', w, flat(x2))

    def unfold(x):
        xp = np.pad(x, ((0, 0), (0, 0), (1, 1), (1, 1)))
        cols = np.stack([xp[:, :, i:i + H, j:j + W_]
                         for i in range(3) for j in range(3)], axis=2)
        return cols.reshape(x.shape[0], x.shape[1] * 9, L)

    def re(t):
        p = t.shape[1]
        return t.reshape(B, A, p, L).transpose(0, 3, 1, 2).reshape(B, L, -1)

    Q = re(unfold(q))
    K = re(unfold(k))
    return Q, K


def _gather_fold(x3, idx):
    """Host epilogue: gather unfold(v) rows by idx and fold back."""
    v = x3.transpose(0, 2, 3, 1, 4, 5).reshape(B * A, C, H, W_)
    vp = np.pad(v, ((0, 0), (0, 0), (1, 1), (1, 1)))
    cols = np.stack([vp[:, :, i:i + H, j:j + W_]
                     for i in range(3) for j in range(3)], axis=2)
    V = cols.reshape(B, A, C * 9, L).transpose(0, 3, 1, 2).reshape(B, L, -1)
    outc = np.take_along_axis(V, idx[:, :, None], axis=1)
    p_v = C * 9
    outc = outc.reshape(B, L, A, p_v).transpose(0, 2, 3, 1)
    outc = outc.reshape(B * A, C, 3, 3, H, W_)
    out = np.zeros((B * A, C, H + 2, W_ + 2), np.float32)
    for i in range(3):
        for j in range(3):
            out[:, :, i:i + H, j:j + W_] += outc[:, :, i, j]
    out = out[:, :, 1:1 + H, 1:1 + W_]
    return np.ascontiguousarray(
        out.reshape(B, AH, AW, C, H, W_).transpose(0, 3, 1, 2, 4, 5))


def _decode_rescore(mx_res, x1, x2, w):
    """Decode top-8 candidate windows per row, rescore exactly, argmax."""
    n_at = (SLAB + 127) // 128
    at_rows = [min(128, SLAB - 128 * t) for t in range(n_at)]
    # candidate windows [B, L, 8]
    cand_w = np.zeros((B, L, 8), np.int64)
    for core in range(NCORES):
        b, r = core // 4, core % 4
        m = mx_res[core].reshape(n_at, 128, 8)
        bits = m.view(np.uint32)
        w_idx = (bits & 0x1FF).astype(np.int64)   # NW=288 < 2^9
        for t in range(n_at):
            rows = at_rows[t]
            n0 = SLAB * r + 128 * t
            cand_w[b, n0:n0 + rows] = w_idx[t, :rows]
    # candidate columns [B, L, 64]
    cols = (cand_w[:, :, :, None] * WIN
            + np.arange(WIN, dtype=np.int64)[None, None, None, :])
    cols = cols.reshape(B, L, 8 * WIN)

    Q, K = _exact_qk(x1, x2, w)
    Kn = np.linalg.norm(K.astype(np.float64), axis=-1)
    idx = np.zeros((B, L), np.int64)
    CHK = 256
    for b in range(B):
        Q64 = Q[b].astype(np.float64)
        K64 = K[b].astype(np.float64)
        for n0 in range(0, L, CHK):
            n1 = min(L, n0 + CHK)
            cc = cols[b, n0:n1]                      # [ch, 64]
            Kc = K64[cc]                             # [ch, 64, 1800]
            sc = np.einsum('nd,nkd->nk', Q64[n0:n1], Kc)
            sc /= Kn[b][cc]
            idx[b, n0:n1] = cc[np.arange(n1 - n0), np.argmax(sc, axis=1)]
    return idx


def _make_in_maps(x1, x2, w):
    import ml_dtypes
    qp, kp, rk = _host_prep(x1, x2, w)
    iota = np.broadcast_to(np.arange(NW, dtype=np.uint32), (128, NW)).copy()
    # identity-shift weights: ish[p, r] = 1 where dest row r reads source
    # partition p (lhsT layout), for the +48/+96 row shifts
    ish = np.zeros((4, 128, 128), np.float32)
    for p in range(48, 128):
        ish[0, p, p - 48] = 1.0       # i1a
    for p in range(0, 48):
        ish[1, p, p + 80] = 1.0       # i1b
    for p in range(96, 128):
        ish[2, p, p - 96] = 1.0       # i2a
    for p in range(0, 96):
        ish[3, p, p + 32] = 1.0       # i2b
    ish = ish.astype(ml_dtypes.bfloat16)
    in_maps = []
    for core in range(NCORES):
        b, r = core // 4, core % 4
        u0 = SLAB * r
        in_maps.append({
            "qpT": np.ascontiguousarray(qp[b][:, :, u0:u0 + USLAB]),
            "kpT": kp[b],
            "rk": np.broadcast_to(rk[b], (128, L)).copy(),
            "iota": iota,
            "ish": ish,
        })
    return in_maps


def kernel(x1, x2, x3, W):
    global _PROG
    sys.path.insert(0, '/opt/trn_rl_repo')
    from concourse.bass_utils import run_bass_kernel_spmd

    x1 = np.asarray(x1, dtype=np.float32)
    x2 = np.asarray(x2, dtype=np.float32)
    x3 = np.asarray(x3, dtype=np.float32)
    w = np.asarray(W, dtype=np.float32)

    in_maps = _make_in_maps(x1, x2, w)
    if _PROG is None:
        _PROG = _build_program()
    res = run_bass_kernel_spmd(_PROG, in_maps, list(range(NCORES)))

    mx_res = [res.results[core]["mx"] for core in range(NCORES)]
    idx = _decode_rescore(mx_res, x1, x2, w)
    return _gather_fold(x3, idx)
